# revision 20
# baseline (speedup 1.0000x reference)
"""Quantized-weight batched linear: out[b,n,m] = sum_k deq(qweight)[n,k] * x[b,k,m].

Strategy (pure fp8e4m3 DoubleRow):
  - Host: dequantize weight (fp32, exact oracle formula), then:
      * weights centered before e4m3 rounding: w8 = e4m3(w - 0.5), so the
        PE sees values in [-0.5, 0.5] (halves fp8 quantization error vs
        [0,1], and removes the weight-DC channel from the x-error term).
      * x rounded to e4m3.
      * the exact rank-1 decomposition  w@x = (w-0.5)@x + 0.5*colsum(x)
        is restored by adding dc[b,m] = 0.5*sum_k x[b,k,m] -- computed on
        host from the TRUE x in fp32 -- during the PSUM drain (DVE
        tensor_tensor add, dc broadcast across partitions).  Computing dc
        from true x (not quantized x) also cancels the dominant x-error
        channel: the error w@(x8-x) through the weight-DC.
    Measured end-to-end rel_err on the oracle inputs: 1.78e-2 (gate 2e-2),
    absmax/scale 8.6e-3.
  - Device (8 cores, data-parallel over batch B=64 -> 8 batches/core):
    all K=1024 contraction via 4 fp8 DoubleRow passes per output tile
    (2 k-chunks of 128 per pass; DR streams at the same ~216ns/pass as a
    single bf16 matmul at FD=512 -> ~2x throughput).  N tiled 8x128,
    M tiled 2x512 (PSUM banks), fp16 output (halves store traffic; host
    casts back to fp32).
  - Pipeline: x loads on the sync HWDGE queue; w loads + output stores on
    scalar.  Warmup matmuls run during the initial DMA window (PE clock
    boost).  Batch 0 opens with a wide 4-n-tile PSUM group so the DMA
    stream builds a lead; batches 1-6 run in pairs (one weight load feeds
    4 consecutive matmuls, keeping the 256-col DoubleRow LDWEIGHTS off the
    critical path); batch 7 tapers [2,2,2,1,1] with the final n-tile in
    three serial m-pieces so the last drains overlap compute.
  - Gather core outputs along batch -> (64, 1024, 1024) fp32.
"""

import numpy as np
import ml_dtypes

N = 1024  # output rows (weight rows)
K = 1024  # reduction dim
M = 1024  # columns of x per batch
NGROUP = 16
GS = K // NGROUP
B = 64
NCORES = 8
BPC = B // NCORES  # batches per core

KDR = 4   # fp8 DoubleRow passes (2 k-chunks of 128 each)

NWARM = 30  # FD=128 PE warmup matmuls (~3.4us cold, covers the cold-DMA window)

_CACHE = {}
LAST_RESULT = None  # BassKernelResults of the most recent run (for profiling)


def _build_nc(bpc=BPC, k=K, n=N, m=M, nwarm=NWARM):
    import concourse.mybir as mybir
    import concourse.tile as tile
    from concourse import bacc

    nt = n // 128   # output-row tiles (PSUM partition dim)
    mt = m // 512   # moving free-dim tiles (one PSUM bank each)
    DR = mybir.MatmulPerfMode.DoubleRow

    nc = bacc.Bacc(None, target_bir_lowering=False, debug=False)
    w8d = nc.dram_tensor("w8", [KDR, 128, 2, n], mybir.dt.float8e4, kind="ExternalInput")
    x8d = nc.dram_tensor("x8", [bpc, KDR, 128, 2, m], mybir.dt.float8e4, kind="ExternalInput")
    dcd = nc.dram_tensor("dc", [bpc, 128, m], mybir.dt.float16, kind="ExternalInput")
    out = nc.dram_tensor("out", [bpc, n, m], mybir.dt.float16, kind="ExternalOutput")

    npg = 2  # n-tiles per PSUM group (4 banks live; 8-bank pool double-buffers)

    with tile.TileContext(nc) as tc:
        with (
            tc.tile_pool(name="wpool", bufs=1) as wpool,
            tc.tile_pool(name="x8pool", bufs=4) as x8pool,
            tc.tile_pool(name="dcpool", bufs=4) as dcpool,
            tc.tile_pool(name="opool", bufs=16) as opool,
            tc.tile_pool(name="spool", bufs=1) as spool,
            tc.tile_pool(name="psum", bufs=8, space="PSUM") as psum_pool,
        ):
            # Warmup: scratch-fed matmuls run while the first DMAs are in
            # flight (PE clock boost takes ~3.4us of continuous busy time).
            # Small scratch (fast memset) + FD=128 warmup matmuls: the
            # warmup starts ~0.5us after the TileContext barrier and the HAM
            # clock-flip completes before the first x slab lands, so the
            # whole real stream runs at 2.4GHz.
            scr = spool.tile([128, 128], mybir.dt.bfloat16, tag="scr", name="scr")
            nc.gpsimd.memset(scr[:], 0.0)
            pwarm = psum_pool.tile([128, 512], mybir.dt.float32, tag="ps", name="pswarm")
            for i in range(nwarm):
                nc.tensor.matmul(pwarm[:, 0:128], scr[:], scr[:], start=True, stop=True)

            # x loads (sync queue): 4 slab DMAs [128, 2, 1024] (2KB rows),
            # dc last (only needed at drain time).
            x8tiles = {}  # b -> fp8 [128, 2*KDR, m] tile
            dctiles = {}  # b -> fp16 [128, m] tile

            def load_x(b):
                t8 = x8pool.tile([128, 2 * KDR, m], mybir.dt.float8e4, tag="x8",
                                 name=f"x8_{b}")
                for j in range(KDR):
                    if b == 0 and j == 0:
                        # split the very first slab so the first matmul only
                        # waits on a 128KB transfer
                        nc.sync.dma_start(out=t8[:, 0:2, 0:512],
                                          in_=x8d[0, 0, :, :, 0:512])
                        nc.sync.dma_start(out=t8[:, 0:2, 512:m],
                                          in_=x8d[0, 0, :, :, 512:m])
                        continue
                    nc.sync.dma_start(
                        out=t8[:, 2 * j:2 * j + 2, :],
                        in_=x8d[b, j, :, :, :],
                    )
                x8tiles[b] = t8
                td = dcpool.tile([128, m], mybir.dt.float16, tag="dc",
                                 name=f"dc_{b}")
                nc.sync.dma_start(out=td[:], in_=dcd[b, :, :])
                dctiles[b] = td

            # w loads ride the scalar queue (only used for stores later).
            # Slab 0 first so the first DoubleRow pass only waits on 256KB.
            w8t = wpool.tile([128, 2 * KDR, n], mybir.dt.float8e4, tag="w8",
                             name="w8t")
            # n-half interleave: all four slabs' n<512 halves (128KB each)
            # load first -- batch 0's wide first group only touches n-tiles
            # 0-3, so every DoubleRow pass of the first group has its
            # weights early even when the cold-DMA path runs slow; the
            # n>=512 halves follow long before group 2 (~16us in) needs them.
            nc.scalar.dma_start(out=w8t[:, 0:2, 0:512], in_=w8d[0, :, :, 0:512])
            load_x(0)
            for j in range(1, KDR):
                nc.scalar.dma_start(out=w8t[:, 2 * j:2 * j + 2, 0:512],
                                    in_=w8d[j, :, :, 0:512])
            for j in range(KDR):
                nc.scalar.dma_start(out=w8t[:, 2 * j:2 * j + 2, 512:n],
                                    in_=w8d[j, :, :, 512:n])

            def dr_mm(pst, b, dj, n0, mlo, mhi, start, stop):
                nc.tensor.matmul(
                    pst,
                    w8t[:, 2 * dj:2 * dj + 2, n0 * 128:(n0 + 1) * 128],
                    x8tiles[b][:, 2 * dj:2 * dj + 2, mlo:mhi],
                    start=start, stop=stop, perf_mode=DR,
                )

            # Batch schedule: batch 0 solo (wide first group builds the DMA
            # lead), batches 1-6 in pairs (a weight load feeds 4 consecutive
            # matmuls), batch 7 solo with the tapered tail.
            sched = [(0,)] + [(i, i + 1) for i in range(1, bpc - 1, 2)] + [(bpc - 1,)]
            prefetch = {0: (1, 2), 1: (3, 4), 2: (5, 6), 3: (7,)}

            for si, bs in enumerate(sched):
                for pb in prefetch.get(si, ()):
                    if pb < bpc:
                        load_x(pb)

                if len(bs) == 2:
                    # Paired batches: groups of 1 n-tile x 2 batches x 2
                    # m-banks (4 PSUM banks live, 8-bank pool double-buffers).
                    for n0 in range(nt):
                        ps = {}
                        for bb in bs:
                            for m0 in range(mt):
                                ps[bb, m0] = psum_pool.tile(
                                    [128, 512], mybir.dt.float32, tag="ps",
                                    name=f"pp{bs[0]}_{n0}_{bb}_{m0}"
                                )
                        for dj in range(KDR):
                            for bb in bs:
                                for m0 in range(mt):
                                    dr_mm(ps[bb, m0][:], bb, dj, n0,
                                          m0 * 512, (m0 + 1) * 512,
                                          start=(dj == 0), stop=(dj == KDR - 1))
                        for bb in bs:
                            ot = opool.tile([128, m], mybir.dt.float16,
                                            tag="o", name=f"op{bb}_{n0}")
                            for m0 in range(mt):
                                nc.vector.tensor_tensor(
                                    ot[:, m0 * 512:(m0 + 1) * 512],
                                    ps[bb, m0][:],
                                    dctiles[bb][:, m0 * 512:(m0 + 1) * 512],
                                    op=mybir.AluOpType.add,
                                )
                            eng = nc.scalar if (n0 + bb) % 2 == 0 else nc.sync
                            eng.dma_start(
                                out=out[bb, n0 * 128:(n0 + 1) * 128, :],
                                in_=ot[:],
                            )
                    continue

                b = bs[0]
                if b == 0:
                    # Wide first group: consumes x at half rate so the DMA
                    # stream builds a lead instead of racing the PE.
                    groups = [4, 2, 2]
                elif b == bpc - 1:
                    groups = [2, 2, 2, 1, 1]
                else:
                    groups = [npg] * (nt // npg)
                n0_base = 0
                for h, gsz in enumerate(groups):
                    final = b == bpc - 1 and h == len(groups) - 1
                    if final:
                        # Final n-tile: three m-pieces run serially so earlier
                        # pieces drain under the later pieces' matmuls.
                        n0 = n0_base
                        pieces = [(0, 512), (512, 384), (896, 128)]
                        for pi, (moff, mw) in enumerate(pieces):
                            pbank = psum_pool.tile(
                                [128, 512], mybir.dt.float32, tag="ps",
                                name=f"psf_{pi}"
                            )
                            pst = pbank[:, 0:mw]
                            for dj in range(KDR):
                                dr_mm(pst, b, dj, n0, moff, moff + mw,
                                      start=(dj == 0), stop=(dj == KDR - 1))
                            ot = opool.tile([128, mw], mybir.dt.float16, tag="o",
                                            name=f"of_{pi}")
                            dst = out[b, n0 * 128:(n0 + 1) * 128, moff:moff + mw]
                            nc.vector.tensor_tensor(
                                ot[:], pst, dctiles[b][:, moff:moff + mw],
                                op=mybir.AluOpType.add,
                            )
                            # split the final drains across both queue rings
                            if pi == 1:
                                nc.sync.dma_start(out=dst, in_=ot[:])
                            else:
                                nc.scalar.dma_start(out=dst, in_=ot[:])
                        n0_base += gsz
                        continue

                    ps = {}
                    for j in range(gsz):
                        for m0 in range(mt):
                            ps[j, m0] = psum_pool.tile(
                                [128, 512], mybir.dt.float32, tag="ps",
                                name=f"ps{b}_{h}_{j}_{m0}"
                            )
                    # k-outer: every x slab is fully consumed on arrival.
                    for dj in range(KDR):
                        for j in range(gsz):
                            for m0 in range(mt):
                                dr_mm(ps[j, m0][:], b, dj, n0_base + j,
                                      m0 * 512, (m0 + 1) * 512,
                                      start=(dj == 0), stop=(dj == KDR - 1))
                    for j in range(gsz):
                        n0 = n0_base + j
                        ot = opool.tile([128, m], mybir.dt.float16,
                                        tag="o", name=f"o{b}_{j}_{h}")
                        for m0 in range(mt):
                            nc.vector.tensor_tensor(
                                ot[:, m0 * 512:(m0 + 1) * 512],
                                ps[j, m0][:],
                                dctiles[b][:, m0 * 512:(m0 + 1) * 512],
                                op=mybir.AluOpType.add,
                            )
                        eng = nc.scalar if (b == 0 or (n0 + b) % 2 == 0) else nc.sync
                        eng.dma_start(
                            out=out[b, n0 * 128:(n0 + 1) * 128, :],
                            in_=ot[:],
                        )
                    n0_base += gsz
    nc.compile()
    return nc


def _dequant_w(qweight, qrange, qmin):
    # Matches reference: w = q * qrange + qmin per (row, group), fp32.
    q = np.asarray(qweight).astype(np.float32).reshape(N, NGROUP, GS)
    qr = np.asarray(qrange).astype(np.float32).reshape(N, NGROUP, 1)
    qm = np.asarray(qmin).astype(np.float32).reshape(N, NGROUP, 1)
    return (q * qr + qm).reshape(N, K)  # (N, K)


def _e4m3(a):
    return np.asarray(a, np.float32).astype(ml_dtypes.float8_e4m3fn)


def _prep_inputs(x, qweight, qrange, qmin):
    """Host-side quantization. Returns device input arrays (weights shared,
    x8/dc sharded by batch outside)."""
    w = _dequant_w(qweight, qrange, qmin)            # (N, K) fp32

    # fp8 weights, centered, slab-contiguous:
    # w8[dj, p, i, n] = e4m3(w[n, (2*dj+i)*128 + p] - 0.5)
    w_shift = np.ascontiguousarray(w.T - 0.5).reshape(KDR, 2, 128, N)
    w8_host = np.ascontiguousarray(
        _e4m3(w_shift).transpose(0, 2, 1, 3))         # (KDR, 128, 2, N)

    xf = np.asarray(x).astype(np.float32)             # (B, K, M)
    x8q = _e4m3(xf)                                   # (B, K, M) e4m3

    # x8 device layout, slab-contiguous: [b, dj, p, i, m]
    x8_host = np.ascontiguousarray(
        x8q.reshape(B, KDR, 2, 128, M).transpose(0, 1, 3, 2, 4))

    # dc[b, m] = 0.5 * colsum(TRUE x): restores the w-centering exactly and
    # cancels the weight-DC component of the x-quantization error.
    dc_bm = 0.5 * xf.sum(axis=1, dtype=np.float32)    # (B, M)
    dc_host = np.ascontiguousarray(
        np.broadcast_to(dc_bm[:, None, :], (B, 128, M))).astype(np.float16)

    return w8_host, x8_host, dc_host


def _ensure_axon_hooks():
    """run_bass_kernel_spmd(trace=True) imports antenv.axon_hooks, which some
    images lack; provide a stub (and register the real NTFF hook if the boot
    package is present) so tracing degrades gracefully instead of crashing."""
    try:
        import antenv.axon_hooks  # noqa: F401
        return
    except ImportError:
        pass
    try:
        import sys
        import types

        import antenv

        mod = types.ModuleType("antenv.axon_hooks")
        mod._hook = None
        mod.set_axon_ntff_profile_hook = lambda h: setattr(mod, "_hook", h)
        mod.get_axon_ntff_profile_hook = lambda: mod._hook
        sys.modules["antenv.axon_hooks"] = mod
        antenv.axon_hooks = mod
        try:
            from trn_agent_boot.trn_boot import _ntff_profile_via_ctypes

            mod._hook = _ntff_profile_via_ctypes("/opt/axon/libaxon_pjrt.so")
        except Exception:
            pass
    except Exception:
        pass


def kernel(x, qweight, qrange, qmin):
    global LAST_RESULT
    _ensure_axon_hooks()
    from concourse.bass_utils import run_bass_kernel_spmd

    w8_host, x8_host, dc_host = _prep_inputs(x, qweight, qrange, qmin)

    if "nc" not in _CACHE:
        _CACHE["nc"] = _build_nc()
    nc = _CACHE["nc"]

    in_maps = [
        {
            "w8": w8_host,
            "x8": np.ascontiguousarray(x8_host[c * BPC:(c + 1) * BPC]),
            "dc": np.ascontiguousarray(dc_host[c * BPC:(c + 1) * BPC]),
        }
        for c in range(NCORES)
    ]
    LAST_RESULT = run_bass_kernel_spmd(nc, in_maps, core_ids=list(range(NCORES)))
    outs = [r["out"] for r in LAST_RESULT.results]
    return np.ascontiguousarray(
        np.concatenate(outs, axis=0)).astype(np.float32, copy=False)
